# revision 5
# baseline (speedup 1.0000x reference)
"""HalfKP input layer (embedding_lookup) on 8 Trainium2 NeuronCores.

Reference computation (B=1024, K=64, F=640, C=256):
    p = piece_positions.reshape(B, 640).astype(f32)          # values in {0,1}
    Wg = input_weights[king_positions]                       # (B, 2, 641, 256)
    out[b] = sum_f p[b,f] * (Wg[b,0,f,:] + Wg[b,1,f,:])
             + Wg[b,0,640,:] + Wg[b,1,640,:] + bias

Strategy — SINGLE launch, channel-sharded (launch fixed cost on this stack
is ~13-16us per launch, so the two-launch host-routed design pays that
twice):
  * Core c owns output channels [32c, 32c+32).  It computes ALL 2048
    (sample, king-slot) pair rows restricted to its 32 channels, so the
    pair combine (rowA + rowB) is core-local — no cross-core traffic.
  * Weight table read stays minimal: each core reads only its 32-channel
    slice of the table (bf16), 2.62 MB/core -> the full table is read
    exactly once in aggregate.  bf16-only weights (no hi/lo split) keep
    max-rel-err ~1e-3, well under the 2e-2 gate.
  * The 2048 pairs are grouped by king into column-contiguous runs packed
    exactly into 4 PSUM bins of 512 columns (groups split at bin
    boundaries).  Compute is transposed: psum[32ch, col] += W[k,ch].T @
    feats, one matmul per (subgroup, feature-chunk); a final K=2 matmul
    per subgroup adds the row-640 extra (all pairs) and bias (slot-A
    pairs only).
  * Features go over the wire as uint8 (0/1) and are cast to bf16 by the
    SWDGE DMA (gpsimd), halving their HBM traffic.
  * Pair combine: one GPSIMD ap_gather pulls rowA|rowB columns into
    pair order, one DVE add produces outT[32, 1024].  The host transposes
    the per-core slices back into (1024, 256) — pure indexing.
"""

import os
from contextlib import ExitStack

import numpy as np
import ml_dtypes

import concourse.bass as bass
import concourse.tile as tile
from concourse import bacc, mybir
from concourse.bass_utils import run_bass_kernel_spmd

B = 1024
K = 64
F = 640
C = 256
NCORES = 8
CH = C // NCORES       # 32 channels per core
FCH = F // 128         # 5 feature chunks of 128
P = 128
NCOL = 2 * B           # 2048 pair columns
BINW = 512             # one PSUM bank of fp32
NBIN = NCOL // BINW    # 4
WCHUNK = 16            # kings per weight DMA chunk

BF16 = ml_dtypes.bfloat16

# Exposed for test harnesses
LAST_RESULTS = []
LAST_EXEC_NS = None

_cache = {}


def _prep(king_positions):
    """Group the 2048 (sample, slot) pairs by king into a 2048-long column
    order, split groups exactly at 512-column PSUM-bank boundaries."""
    kings = np.asarray(king_positions).astype(np.int64)  # (B, 2)
    groups = [[] for _ in range(K)]
    for b in range(B):
        groups[kings[b, 0]].append((b, 0))
        groups[kings[b, 1]].append((b, 1))

    order = []       # col -> (b, s)
    subgroups = []   # (king, c0, n)
    col = 0
    for k in range(K):
        g = groups[k]
        i = 0
        while i < len(g):
            room = BINW - (col % BINW)
            n = min(len(g) - i, room)
            subgroups.append((k, col, n))
            order.extend(g[i : i + n])
            i += n
            col += n
    assert col == NCOL

    pos = np.empty((B, 2), dtype=np.int64)
    for c, (b, s) in enumerate(order):
        pos[b, s] = c
    return order, tuple(subgroups), pos


def _build(subgroups):
    nc = bacc.Bacc(
        "TRN2", target_bir_lowering=False, debug=False, num_devices=NCORES
    )
    dt = mybir.dt

    w_in = nc.dram_tensor("w_in", [P, K, FCH, CH], dt.bfloat16, kind="ExternalInput")
    feats_u8 = nc.dram_tensor(
        "feats_u8", [P, FCH, NCOL], dt.uint8, kind="ExternalInput"
    )
    eb = nc.dram_tensor("eb", [2, K * CH], dt.bfloat16, kind="ExternalInput")
    flags = nc.dram_tensor("flags", [2, NCOL], dt.bfloat16, kind="ExternalInput")
    idx = nc.dram_tensor("idx", [CH, NCOL // 16], dt.int16, kind="ExternalInput")
    outT = nc.dram_tensor("outT", [CH, B], dt.float32, kind="ExternalOutput")

    with tile.TileContext(nc) as tc, ExitStack() as ctx:
        const_pool = ctx.enter_context(tc.tile_pool(name="const", bufs=1))
        psum_pool = ctx.enter_context(tc.tile_pool(name="psum", bufs=1, space="PSUM"))

        # small tensors first (SWDGE queue, overlaps HWDGE weight stream)
        idx_sb = const_pool.tile([CH, NCOL // 16], dt.int16)
        nc.gpsimd.dma_start(out=idx_sb[:], in_=idx.ap())
        flags_sb = const_pool.tile([2, NCOL], dt.bfloat16)
        nc.gpsimd.dma_start(out=flags_sb[:], in_=flags.ap())
        eb_sb = const_pool.tile([2, K * CH], dt.bfloat16)
        nc.gpsimd.dma_start(out=eb_sb[:], in_=eb.ap())

        # weights: 4 chunks of 16 kings on the two HWDGE queues
        w_sb = []
        for wc in range(K // WCHUNK):
            t = const_pool.tile([P, WCHUNK * FCH * CH], dt.bfloat16, tag="w")
            eng = nc.sync if wc % 2 == 0 else nc.scalar
            eng.dma_start(
                out=t[:],
                in_=w_in[:, wc * WCHUNK : (wc + 1) * WCHUNK, :, :].rearrange(
                    "p k f c -> p (k f c)"
                ),
            )
            w_sb.append(t)

        # features: 4 bin-sized pieces, uint8 -> bf16 cast during DMA (SWDGE)
        f_sb = []
        for bin_ in range(NBIN):
            t = const_pool.tile([P, FCH * BINW], dt.bfloat16, tag="f")
            nc.gpsimd.dma_start(
                out=t[:].rearrange("p (f c) -> p f c", f=FCH),
                in_=feats_u8[:, :, bin_ * BINW : (bin_ + 1) * BINW],
            )
            f_sb.append(t)

        psum_t = [
            psum_pool.tile([P, BINW], dt.float32, space="PSUM", name=f"acc{i}")
            for i in range(NBIN)
        ]

        for k, c0, n in subgroups:
            bin_ = c0 // BINW
            off = c0 % BINW
            wc, kk = divmod(k, WCHUNK)
            for ch in range(FCH):
                nc.tensor.matmul(
                    out=psum_t[bin_][0:CH, off : off + n],
                    lhsT=w_sb[wc][:, (kk * FCH + ch) * CH : (kk * FCH + ch + 1) * CH],
                    rhs=f_sb[bin_][:, ch * BINW + off : ch * BINW + off + n],
                    start=(ch == 0),
                    stop=False,
                )
            # row-640 extra for every pair + bias on slot-A pairs (K=2)
            nc.tensor.matmul(
                out=psum_t[bin_][0:CH, off : off + n],
                lhsT=eb_sb[0:2, k * CH : (k + 1) * CH],
                rhs=flags_sb[0:2, c0 : c0 + n],
                start=False,
                stop=True,
            )

        rows_sb = const_pool.tile([CH, NCOL], dt.float32)
        for bin_ in range(NBIN):
            nc.vector.tensor_copy(
                rows_sb[:, bin_ * BINW : (bin_ + 1) * BINW], psum_t[bin_][0:CH, :]
            )

        gat_sb = const_pool.tile([CH, NCOL], dt.float32)
        nc.gpsimd.ap_gather(
            out_ap=gat_sb[:],
            in_ap=rows_sb[:],
            idxs_ap=idx_sb[:],
            channels=CH,
            num_elems=NCOL,
            d=1,
            num_idxs=NCOL,
        )
        outT_sb = const_pool.tile([CH, B], dt.float32)
        nc.vector.tensor_add(outT_sb[:], gat_sb[:, 0:B], gat_sb[:, B : 2 * B])
        nc.sync.dma_start(out=outT.ap(), in_=outT_sb[:])

    nc.compile()
    return nc


def kernel(piece_positions, king_positions, input_weights, bias):
    global LAST_RESULTS, LAST_EXEC_NS

    p_flat = np.asarray(piece_positions).reshape(B, F)
    w_full = np.ascontiguousarray(np.asarray(input_weights), dtype=np.float32)
    bias_np = np.asarray(bias, dtype=np.float32)

    order, subgroups, pos = _prep(king_positions)

    if subgroups not in _cache:
        _cache[subgroups] = _build(subgroups)
    nc = _cache[subgroups]

    w_bf = w_full.astype(BF16)  # (K, 641, C)

    # features in pair-column order: (2048, 640) -> (128, 5, 2048) uint8
    bs = np.array([b for b, _ in order], dtype=np.int64)
    feats = (
        p_flat[bs].astype(np.uint8).reshape(NCOL, FCH, 128).transpose(2, 1, 0)
    )
    feats = np.ascontiguousarray(feats)

    # flags: row0 = 1 (row-640 extra), row1 = slot-A indicator (bias once)
    flags = np.zeros((2, NCOL), dtype=np.float32)
    flags[0, :] = 1.0
    flags[1, :] = np.array([1.0 if s == 0 else 0.0 for _, s in order])
    flags = flags.astype(BF16)

    # gather indices: [idxA | idxB], wrapped in 16 partitions, replicated
    idxall = np.concatenate([pos[:, 0], pos[:, 1]]).astype(np.int16)  # (2048,)
    idx_w = np.zeros((CH, NCOL // 16), dtype=np.int16)
    wrap = idxall.reshape(NCOL // 16, 16).T  # [p, col]
    for blk in range(CH // 16):
        idx_w[blk * 16 : (blk + 1) * 16, :] = wrap

    in_maps = []
    for c in range(NCORES):
        chs = slice(c * CH, (c + 1) * CH)
        w_c = (
            w_bf[:, :F, chs].reshape(K, FCH, 128, CH).transpose(2, 0, 1, 3)
        )  # (128, K, FCH, CH)
        eb_c = np.zeros((2, K, CH), dtype=np.float32)
        eb_c[0] = w_full[:, F, chs]
        eb_c[1] = bias_np[chs][None, :]
        in_maps.append(
            {
                "w_in": np.ascontiguousarray(w_c),
                "feats_u8": feats,
                "eb": np.ascontiguousarray(eb_c.reshape(2, K * CH)).astype(BF16),
                "flags": flags,
                "idx": idx_w,
            }
        )

    do_trace = bool(int(os.environ.get("KERNEL_TRACE", "0")))
    trace_kw = dict(
        trace=do_trace, trace_cores=list(range(NCORES)) if do_trace else None
    )

    res = run_bass_kernel_spmd(nc, in_maps, list(range(NCORES)), **trace_kw)

    LAST_RESULTS = [res]
    LAST_EXEC_NS = res.exec_time_ns

    out = np.empty((B, C), dtype=np.float32)
    for c in range(NCORES):
        out[:, c * CH : (c + 1) * CH] = res.results[c]["outT"].T
    return out
